# revision 3
# baseline (speedup 1.0000x reference)
"""Trainium2 Bass kernel for nn_Causal_Kron_Block_MLP.

Reference computation (B=4, L=2048, D=1024, H=16, HD=64):
    y1 = x @ W1a.T                                   # [B,L,D]
    z  = relu(einsum('hlm,bhmd->bhld', tril(mat2a), split_heads(y1)))
    y2 = merge_heads(z) @ W1b.T
    w  = einsum('hlm,bhmd->bhld', tril(mat2b), split_heads(y2))
    out = einsum('bhld,hde->ble', w, w_out)

Sharding: 8 cores, head-parallel — core c owns heads (2c, 2c+1).
The kernel is one software pipeline ordered by sequence chunk
(lc = 512-row l-blocks):

  phase A, per lc: s1 (x @ W1a.T for the 4 batches' lc rows, with
    fused PE transpose) -> s2 (causal tril_a chunk, all m <= lc) ->
    relu -> AllGather chunk lc (z for all heads, rows (b, lc)).
    The 4 chunked AllGathers overlap later chunks' compute; they are
    the only collectives (a serial CC chain is the pipeline's floor,
    so everything else is kept point-local).
  phase B, per lc: s3 (W1b over the gathered z chunk) -> s4 (tril_b)
    -> s5 (partial head-sum out rows for this chunk, fp16 scaled).
    The host sums the 8 partial outputs.

All heavy DMA sources are host-pre-tiled so every descriptor moves
>=1KB contiguous runs per partition (x tiles 8KB, tril full-blocks
4KB).  Causality: tril blocks above the diagonal are never loaded;
diagonal blocks are packed host-side with their zero prefix stripped.
All matmuls run in fp16 with f32 PSUM accumulation; measured
end-to-end relative error vs the f32 reference is ~1e-3.  out_part is
fp16 scaled by 1024 (values ~1e-5 would be fp16-subnormal unscaled);
the host sums in f32 and rescales.
"""

import numpy as np

import concourse.bass as bass
import concourse.mybir as mybir
import concourse.tile as tile
from concourse import bacc
from concourse.bass_utils import run_bass_kernel_spmd

B, L, D, H, HD = 4, 2048, 1024, 16, 64
NCORES = 8
R = B * L               # 8192 global rows
NB = 512                # moving free-dim per matmul
N_RB = R // NB          # 16 row-blocks of 512
N_KB_D = D // 128       # 8 k-blocks over model dim
N_MB = L // 128         # 16 m-blocks of 128 per batch
N_LB = L // NB          # 4 l-blocks of 512 per batch
FG_BASE = {1: 0, 2: 1, 3: 3}   # lb -> first full-group index (lb groups)
N_FG = 6                # total full 4x128-m-block groups per h_rel
COL_OFF = [0, 512, 896, 1152]  # packed diag block col offsets (w=512..128)
OUT_SCALE = 1024.0
F32 = mybir.dt.float32
FP16 = mybir.dt.float16

_NC_CACHE = {}


def build_nc():
    """Build the single-NEFF SPMD kernel (same program on all 8 cores)."""
    nc = bacc.Bacc(None, target_bir_lowering=False)

    xT_t = nc.dram_tensor("xT_t", [N_RB, 128, N_KB_D * NB], FP16,
                          kind="ExternalInput")
    w1aT = nc.dram_tensor("w1aT", [128, D], FP16, kind="ExternalInput")
    w1bT = nc.dram_tensor("w1bT", [128, D], FP16, kind="ExternalInput")
    trilAF = nc.dram_tensor("trilAF", [2, N_FG, 128, 4 * NB], FP16,
                            kind="ExternalInput")
    trilAD = nc.dram_tensor("trilAD", [2, N_LB, 128, 1280], FP16,
                            kind="ExternalInput")
    trilBF = nc.dram_tensor("trilBF", [2, N_FG, 128, 4 * NB], FP16,
                            kind="ExternalInput")
    trilBD = nc.dram_tensor("trilBD", [2, N_LB, 128, 1280], FP16,
                            kind="ExternalInput")
    wout = nc.dram_tensor("wout", [128, D], FP16, kind="ExternalInput")
    ident_in = nc.dram_tensor("ident", [128, 128], FP16, kind="ExternalInput")
    out_part = nc.dram_tensor("out_part", [R, D], FP16,
                              kind="ExternalOutput")

    with tile.TileContext(nc) as tc:
        with (
            tc.tile_pool(name="persist", bufs=1) as persist,
            tc.tile_pool(name="xin", bufs=3) as xin,
            tc.tile_pool(name="tfa", bufs=2) as tfa,
            tc.tile_pool(name="tda", bufs=2) as tda,
            tc.tile_pool(name="tfb", bufs=2) as tfb,
            tc.tile_pool(name="tdb", bufs=2) as tdb,
            tc.tile_pool(name="zap", bufs=2) as zap,
            tc.tile_pool(name="ytp", bufs=4) as ytp,
            tc.tile_pool(name="stp", bufs=6) as stp,
            tc.tile_pool(name="ostp", bufs=3) as ostp,
            tc.tile_pool(name="psmm", bufs=5, space="PSUM") as psmm,
            tc.tile_pool(name="pstr", bufs=3, space="PSUM") as pstr,
            tc.tile_pool(name="dram", bufs=1, space="DRAM") as dram,
        ):
            ident = persist.tile([128, 128], FP16, tag="ident")
            nc.sync.dma_start(out=ident[:], in_=ident_in[:])
            w1aT_sb = persist.tile([128, D], FP16, tag="w1aT")
            nc.sync.dma_start(out=w1aT_sb[:], in_=w1aT[:])
            w1bT_sb = persist.tile([128, D], FP16, tag="w1bT")
            wout_sb = persist.tile([128, D], FP16, tag="wout")

            y1mT = persist.tile([128, 2 * R], FP16, tag="y1mT")
            y2mT = persist.tile([128, 2 * R], FP16, tag="y2mT")
            wT_sb = persist.tile([128, R], FP16, tag="wT_sb")

            z_in = [
                dram.tile([128, B * NB], FP16, tag=f"z_in{lc}",
                          name=f"z_in{lc}")
                for lc in range(N_LB)
            ]
            z_all = [
                dram.tile([NCORES * 128, B * NB], FP16, tag=f"z_all{lc}",
                          name=f"z_all{lc}", addr_space="Shared")
                for lc in range(N_LB)
            ]

            def lin_rb(scope, rb, wsb, src_tile, dstmT):
                """One 512-row block of stage 1/3 with fused PE transpose:
                dstmT[(h_rel,p,mb) 128-col blocks][m-part, (j,d)]."""
                b, lc = rb // N_LB, rb % N_LB
                p, j = b // 2, b % 2
                ps = psmm.tile([128, NB], F32, tag="ps_mm",
                               name=f"ps_{scope}_{rb}")
                for kb in range(N_KB_D):
                    nc.tensor.matmul(
                        ps[:],
                        wsb[:, kb * 128:(kb + 1) * 128],
                        src_tile[:, kb * NB:(kb + 1) * NB],
                        start=(kb == 0),
                        stop=(kb == N_KB_D - 1),
                    )
                yt = ytp.tile([128, NB], FP16, tag="yt",
                              name=f"yt_{scope}_{rb}")
                nc.scalar.activation(
                    yt[:], ps[:], mybir.ActivationFunctionType.Copy
                )
                for ml in range(NB // 128):
                    mb = lc * (NB // 128) + ml
                    pst = pstr.tile([128, 128], FP16, tag="ps_tr")
                    nc.tensor.transpose(
                        pst[:], yt[:, ml * 128:(ml + 1) * 128], ident[:]
                    )
                    for h_rel in range(2):
                        off = ((h_rel * 2 + p) * N_MB + mb) * 128
                        nc.vector.tensor_copy(
                            dstmT[:, off + j * HD: off + (j + 1) * HD],
                            pst[:, h_rel * HD:(h_rel + 1) * HD],
                        )

            def tril_block(scope, trilF, trilD, fpool, dpool, srcmT,
                           h_rel, lb, drain):
                """Stage 2/4 chunk: z.T[(j,d), l-cols of lb] for one head,
                accumulating over all m-blocks <= diag (causal)."""
                pss = [
                    psmm.tile([128, NB], F32, tag="ps_mm",
                              name=f"ps_{scope}_{h_rel}_{lb}_{p}")
                    for p in range(2)
                ]
                for g in range(lb):
                    tb = fpool.tile([128, 4 * NB], FP16, tag="tf",
                                    name=f"tf_{scope}_{h_rel}_{lb}_{g}")
                    nc.sync.dma_start(out=tb[:],
                                      in_=trilF[h_rel, FG_BASE[lb] + g])
                    for mi in range(4):
                        mb = 4 * g + mi
                        for p in range(2):
                            off = ((h_rel * 2 + p) * N_MB + mb) * 128
                            nc.tensor.matmul(
                                pss[p][:],
                                srcmT[:, off:off + 128],
                                tb[:, mi * NB:(mi + 1) * NB],
                                start=(mb == 0),
                                stop=False,
                            )
                td = dpool.tile([128, 1280], FP16, tag="td",
                                name=f"td_{scope}_{h_rel}_{lb}")
                nc.sync.dma_start(out=td[:], in_=trilD[h_rel, lb])
                for i in range(4):
                    mb = lb * 4 + i
                    w = NB - i * 128
                    for p in range(2):
                        off = ((h_rel * 2 + p) * N_MB + mb) * 128
                        nc.tensor.matmul(
                            pss[p][:, i * 128:NB],
                            srcmT[:, off:off + 128],
                            td[:, COL_OFF[i]:COL_OFF[i] + w],
                            start=(mb == 0),
                            stop=(i == 3),
                        )
                for p in range(2):
                    drain(h_rel, p, lb, pss[p])

            # ================= phase A ==================================
            def z_drain(h_rel, p, lb, ps):
                st = stp.tile([128, NB], FP16, tag="zst",
                              name=f"zst_{h_rel}_{p}_{lb}")
                nc.scalar.activation(
                    st[:], ps[:], mybir.ActivationFunctionType.Relu
                )
                for jb in range(2):
                    bb = 2 * p + jb
                    nc.sync.dma_start(
                        out=z_in[lb][h_rel * HD:(h_rel + 1) * HD,
                                     bb * NB:(bb + 1) * NB],
                        in_=st[jb * HD:(jb + 1) * HD, :],
                    )

            for lc in range(N_LB):
                with nc.named_scope(f"s1c{lc}"):
                    for b in range(B):
                        rb = b * N_LB + lc
                        xt = xin.tile([128, N_KB_D * NB], FP16, tag="x_blk",
                                      name=f"x_{rb}")
                        nc.sync.dma_start(out=xt[:], in_=xT_t[rb])
                        lin_rb("s1", rb, w1aT_sb, xt, y1mT)
                with nc.named_scope(f"s2c{lc}"):
                    for h_rel in range(2):
                        tril_block("s2", trilAF, trilAD, tfa, tda, y1mT,
                                   h_rel, lc, z_drain)
                nc.gpsimd.collective_compute(
                    "AllGather",
                    mybir.AluOpType.bypass,
                    replica_groups=[list(range(NCORES))],
                    ins=[z_in[lc].opt()],
                    outs=[z_all[lc].opt()],
                )
                if lc == 0:
                    # phase-B weights: emitted here so the DMAs land during
                    # phase A's slack, well before s3/s5 need them
                    nc.sync.dma_start(out=w1bT_sb[:], in_=w1bT[:])
                    nc.sync.dma_start(out=wout_sb[:], in_=wout[:])

            # ================= phase B ==================================
            def w_drain(h_rel, p, lb, ps):
                st = stp.tile([128, NB], FP16, tag="wst",
                              name=f"wst_{h_rel}_{p}_{lb}")
                nc.scalar.activation(
                    st[:], ps[:], mybir.ActivationFunctionType.Copy
                )
                for jb in range(2):
                    bb = 2 * p + jb
                    nc.sync.dma_start(
                        out=wT_sb[h_rel * HD:(h_rel + 1) * HD,
                                  bb * L + lb * NB: bb * L + (lb + 1) * NB],
                        in_=st[jb * HD:(jb + 1) * HD, :],
                    )

            for lc in range(N_LB):
                with nc.named_scope(f"s3c{lc}"):
                    for b in range(B):
                        rb = b * N_LB + lc
                        zt = zap.tile([128, N_KB_D * NB], FP16, tag="z_blk",
                                      name=f"z3_{rb}")
                        nc.sync.dma_start(
                            out=zt[:].rearrange("p (g n) -> p g n",
                                                g=N_KB_D),
                            in_=z_all[lc][:, b * NB:(b + 1) * NB].rearrange(
                                "(g p) n -> p g n", p=128),
                        )
                        lin_rb("s3", rb, w1bT_sb, zt, y2mT)
                with nc.named_scope(f"s4c{lc}"):
                    for h_rel in range(2):
                        tril_block("s4", trilBF, trilBD, tfb, tdb, y2mT,
                                   h_rel, lc, w_drain)
                # s5 for this chunk: partial head-sum rows, fp16 scaled
                with nc.named_scope(f"s5c{lc}"):
                    for b in range(B):
                        base = b * L + lc * NB
                        ost = ostp.tile([128, D], FP16, tag="out_stage",
                                        name=f"ost_{b}_{lc}")
                        for u in range(NB // 128):
                            for eh in range(2):
                                ps = psmm.tile(
                                    [128, NB], F32, tag="ps_mm",
                                    name=f"ps5_{b}_{lc}_{u}_{eh}",
                                )
                                nc.tensor.matmul(
                                    ps[:],
                                    wT_sb[:, base + u * 128:
                                          base + (u + 1) * 128],
                                    wout_sb[:, eh * NB:(eh + 1) * NB],
                                    start=True,
                                    stop=True,
                                )
                                nc.scalar.activation(
                                    ost[:, eh * NB:(eh + 1) * NB],
                                    ps[:],
                                    mybir.ActivationFunctionType.Copy,
                                    scale=OUT_SCALE,
                                )
                            nc.sync.dma_start(
                                out=out_part[base + u * 128:
                                             base + (u + 1) * 128, :],
                                in_=ost[:],
                            )

    nc.finalize()
    return nc


def _tril_tiles(mat_h):
    """Host pre-tiling of one head's tril matrix (transposed, fp16):
    full groups [N_FG][128, 4*NB] and packed diag [N_LB][128, 1280]."""
    T = np.tril(mat_h).T.astype(np.float16)      # [L, L], upper (m <= l)
    F = np.zeros((N_FG, 128, 4 * NB), np.float16)
    for lb in range(1, N_LB):
        for g in range(lb):
            blk = T[g * NB:(g + 1) * NB, lb * NB:(lb + 1) * NB]
            F[FG_BASE[lb] + g] = (
                blk.reshape(4, 128, NB).transpose(1, 0, 2).reshape(128, 4 * NB)
            )
    Dg = np.zeros((N_LB, 128, 1280), np.float16)
    for lb in range(N_LB):
        for i in range(4):
            mb = lb * 4 + i
            w = NB - i * 128
            Dg[lb][:, COL_OFF[i]:COL_OFF[i] + w] = T[
                mb * 128:(mb + 1) * 128, lb * NB + i * 128:(lb + 1) * NB
            ]
    return F, Dg


def prep_in_maps(x, W1a, W1b, mat2a, mat2b, w_out):
    xT = np.ascontiguousarray(x.reshape(R, D).T).astype(np.float16)
    xT_t = np.ascontiguousarray(
        xT.reshape(N_KB_D, 128, N_RB, NB).transpose(2, 1, 0, 3)
    ).reshape(N_RB, 128, N_KB_D * NB)
    ident = np.eye(128, dtype=np.float16)
    # k order for the gathered z: (rank, h_rel, d) -> head 2*rank + h_rel
    k_idx = np.array(
        [(2 * rank + h_rel) * HD + dd
         for rank in range(NCORES) for h_rel in range(2) for dd in range(HD)]
    )

    def tile_w(Wc):          # [128 out, D kin] -> [128 p, (g, 128 out)]
        return np.ascontiguousarray(
            Wc.T.reshape(N_KB_D, 128, 128).transpose(1, 0, 2)
        ).reshape(128, D).astype(np.float16)

    in_maps = []
    for c in range(NCORES):
        heads = [2 * c, 2 * c + 1]
        W1b_c = W1b[128 * c:128 * (c + 1), :][:, k_idx]
        tA = [_tril_tiles(mat2a[h]) for h in heads]
        tB = [_tril_tiles(mat2b[h]) for h in heads]
        in_maps.append(
            {
                "xT_t": xT_t,
                "w1aT": tile_w(W1a[128 * c:128 * (c + 1), :]),
                "w1bT": tile_w(W1b_c),
                "trilAF": np.stack([t[0] for t in tA]),
                "trilAD": np.stack([t[1] for t in tA]),
                "trilBF": np.stack([t[0] for t in tB]),
                "trilBD": np.stack([t[1] for t in tB]),
                "wout": w_out[heads].reshape(128, D).astype(np.float16),
                "ident": ident,
            }
        )
    return in_maps


def kernel(x, W1a, W1b, mat2a, mat2b, w_out):
    x = np.asarray(x, dtype=np.float32)
    W1a = np.asarray(W1a, dtype=np.float32)
    W1b = np.asarray(W1b, dtype=np.float32)
    mat2a = np.asarray(mat2a, dtype=np.float32)
    mat2b = np.asarray(mat2b, dtype=np.float32)
    w_out = np.asarray(w_out, dtype=np.float32)

    if "nc" not in _NC_CACHE:
        _NC_CACHE["nc"] = build_nc()
    nc = _NC_CACHE["nc"]

    in_maps = prep_in_maps(x, W1a, W1b, mat2a, mat2b, w_out)
    res = run_bass_kernel_spmd(nc, in_maps, core_ids=list(range(NCORES)))
    out = np.zeros((R, D), np.float32)
    for c in range(NCORES):
        out += res.results[c]["out_part"].astype(np.float32)
    out *= 1.0 / OUT_SCALE
    return out.reshape(B, L, D)


if __name__ == "__main__":
    rng = np.random.default_rng(0)
    inputs = {
        "x": rng.standard_normal((B, L, D), dtype=np.float32),
        "W1a": rng.standard_normal((D, D), dtype=np.float32) / D,
        "W1b": rng.standard_normal((D, D), dtype=np.float32) / D,
        "mat2a": rng.standard_normal((H, L, L), dtype=np.float32) / 32,
        "mat2b": rng.standard_normal((H, L, L), dtype=np.float32) / 32,
        "w_out": rng.standard_normal((H, HD, D), dtype=np.float32) / D,
    }
    out = kernel(**inputs)
    print("kernel ran, out shape", out.shape)
